# revision 23
# baseline (speedup 1.0000x reference)
"""APKDA loss (pool+normalize -> SmoothAP) as two distributed Bass launches on
8 TRN2 NeuronCores.

Math restructuring vs the reference:
  - Only the diagonal class-blocks of sim_all_rk are ever used, so per query q
    we need rank sums only over its 16 same-class columns j:
        r_all[q,j] = 1 + sum_k relu(S[q,k] - S[q,j])   (k over all 512 columns)
        r_pos[q,j] = 1 + sum_k relu(Sg[q,k] - Sg[q,j]) (k over the 16-group)
    with Sg the own-class block of S.  The eye-masks in the reference only
    kill k==j terms whose relu is 0 anyway.
  - L2-normalizing the hw-sum equals normalizing the hw-mean (scale cancels).

Sharding: batch-parallel.  Core m owns 4 classes = outputs[32m:32m+32] and
targets[32m:32m+32] (6.4MB of the 51.4MB input).

Phase 1 (memory-bound): each core sum-pools its 6.4MB shard over the 7x7
window and returns the raw [128(g,b), 128] pooled sums per branch as bf16.
Host: normalize rows, transpose to feature-major, interleave to reference row
order, concatenate all cores -> full f^T (512KB bf16).
Phase 2: every core gets the full f^T, its own 64 columns duplicated (so
one matmul writes S onto all 128 psum partitions), and the host-computed
own-class Gram diag Sg (bf16-rounded like Sb, so the k==j relu terms cancel);
each core computes its [64, 512] slice of S and the raw rank sums; the host
applies the +1/division/total.

A single-launch variant with an in-kernel AllGather measured 133.7us:
collectives on this runtime have a ~40-80us latency floor regardless of size,
so the f exchange goes through the host instead (two NEFF launches at ~13us
fixed overhead each).  Input DMA tops out at ~205 GB/s/core here no matter
the descriptor path (sync/scalar/gpsimd) or shape, so phase 1 is pinned at
~31us of DMA + overheads.

Measured (neuron-profile exec_time_ns, core 0): phase 1 ~44.5us + phase 2
~23.5us = ~68us total (the shared machine drifts between a fast and a ~15%
slower mode), rel err vs the f32 reference 5.5e-6.  Phase 1 is at its local
floor: ~7us exec preamble + ~31us DMA at the ~205 GB/s cap + ~3us pooling
tail + ~4us Tile drain.  Remaining known headroom: replacing the two
launches with one launch + remote_dma cross-core exchange would save one
~13us launch overhead; Tile serializes any two engines touching the same
PSUM bank (even read-read), which is why S is staged to SBUF before the
rank loop.
"""

import numpy as np
import ml_dtypes

import concourse.bass as bass
import concourse.bacc as bacc
import concourse.mybir as mybir
import concourse.tile as tile
from concourse.bass_utils import run_bass_kernel_spmd

F32 = mybir.dt.float32
BF16 = mybir.dt.bfloat16
NCORES = 8
BATCH, FEAT, HW = 256, 512, 49
BPC = BATCH // NCORES          # 32 batch rows per branch per core
GROUP, B2 = 16, 512

# j-slot split of the 8 (two-j-per-instruction) rank iterations
DVE_SLOTS = 4                  # slots 0..3 on VectorE, 4..7 on ScalarE

# pooling chunk widths (c_local units); outputs loads first, targets' last
# chunk is small so the pooling tail after the final DMA is short
O_CHUNKS = [64, 64]
T_CHUNKS = [56, 56, 8, 8]


def build_phase1(dbg=None):
    """Sum-pool the shard; out: p_o / p_t bf16 [128(g,b), 128 c_local]."""
    nc = bacc.Bacc("TRN2", target_bir_lowering=False, debug=False,
                   num_devices=NCORES)
    f32 = F32
    AX = mybir.AxisListType
    x_out = nc.dram_tensor("x_out", [BPC, FEAT, HW], f32, kind="ExternalInput")
    x_tgt = nc.dram_tensor("x_tgt", [BPC, FEAT, HW], f32, kind="ExternalInput")
    po_d = nc.dram_tensor("p_o", [128, 128], BF16, kind="ExternalOutput")
    pt_d = nc.dram_tensor("p_t", [128, 128], BF16, kind="ExternalOutput")

    with tile.TileContext(nc) as tc, tc.tile_pool(name="sb", bufs=1) as sb:
        xo = sb.tile([128, 6272], f32, tag="xo")
        xt = sb.tile([128, 6272], f32, tag="xt")
        pooled_o = sb.tile([128, 128], BF16, tag="pooled_o")
        pooled_t = sb.tile([128, 128], BF16, tag="pooled_t")

        # partition p = 32g + b; row (g,b) holds x[b, 128g:128g+128, :] flat.
        # g=0,1 (partitions 0-63) ride the sync HWDGE ring, g=2,3 ride scalar,
        # which spreads the load over all 16 SDMA engines.  The reduce
        # accumulates in f32 internally and rounds once on the bf16 store, and
        # all but the last columns are shipped out before the last chunk lands.
        def load_chunks(t_, x_, p_, p_d, widths):
            c0 = 0
            for w in widths:
                for g in range(4):
                    eng = nc.sync if g < 2 else nc.scalar
                    eng.dma_start(
                        t_[32 * g:32 * (g + 1), 49 * c0:49 * (c0 + w)],
                        x_.ap()[:, g * 128 + c0:g * 128 + c0 + w, :])
                with nc.allow_low_precision("f32 accumulate, single bf16 round"):
                    nc.vector.reduce_sum(
                        p_[:, c0:c0 + w],
                        t_[:, 49 * c0:49 * (c0 + w)].rearrange(
                            "p (c h) -> p c h", h=HW),
                        axis=AX.X)
                c0 += w
            c1 = c0 - widths[-1]
            nc.sync.dma_start(p_d.ap()[0:64, 0:c1], p_[0:64, 0:c1])
            nc.scalar.dma_start(p_d.ap()[64:128, 0:c1], p_[64:128, 0:c1])
            nc.sync.dma_start(p_d.ap()[0:64, c1:c0], p_[0:64, c1:c0])
            nc.scalar.dma_start(p_d.ap()[64:128, c1:c0], p_[64:128, c1:c0])

        load_chunks(xo, x_out, pooled_o, po_d, O_CHUNKS)
        load_chunks(xt, x_tgt, pooled_t, pt_d, T_CHUNKS)
    nc.compile()
    return nc


def build_phase2(dbg=None):
    """S slice + raw rank sums from replicated bf16 f^T.
    in: fT_all [4,128,512], fT_own [4,128,128] (bf16, reference col order;
    own block's 64 columns duplicated so the PE emits S on 128 partitions);
    out: racc [128, 16] f32 (cols 0-7 r_all slots, 8-15 r_pos slots)."""
    nc = bacc.Bacc("TRN2", target_bir_lowering=False, debug=False,
                   num_devices=NCORES)
    f32 = F32
    AF = mybir.ActivationFunctionType
    ALU = mybir.AluOpType
    fT_all = nc.dram_tensor("fT_all", [4, 128, 512], BF16, kind="ExternalInput")
    fT_own = nc.dram_tensor("fT_own", [4, 128, 128], BF16, kind="ExternalInput")
    sg_in = nc.dram_tensor("sg", [64, 16], F32, kind="ExternalInput")
    out_d = nc.dram_tensor("out", [128, 16], f32, kind="ExternalOutput")

    with tile.TileContext(nc) as tc, (
            tc.tile_pool(name="sb", bufs=1)) as sb, (
            tc.tile_pool(name="ps", bufs=1, space="PSUM")) as ps:
        ccin = sb.tile([128, 512], BF16, tag="ccin")   # free = (g, col-dup)
        rhs = sb.tile([128, 2048], BF16, tag="rhs")    # free = (g, key)
        # sg first (tiny, no deps) so B8/r_pos are ready long before S;
        # own block next; even/odd partition halves ride the two HWDGE rings
        Sg = sb.tile([64, 16], F32, tag="Sg")
        nc.sync.dma_start(Sg[:, :], sg_in.ap())
        nc.sync.dma_start(
            ccin[0:64, :].rearrange("p (g n) -> p g n", g=4),
            fT_own.ap()[:, 0:64, :].rearrange("g p n -> p g n"))
        nc.scalar.dma_start(
            ccin[64:128, :].rearrange("p (g n) -> p g n", g=4),
            fT_own.ap()[:, 64:128, :].rearrange("g p n -> p g n"))
        for g in range(4):
            nc.sync.dma_start(rhs[0:64, 512 * g:512 * (g + 1)],
                              fT_all.ap()[g, 0:64, :])
            nc.scalar.dma_start(rhs[64:128, 512 * g:512 * (g + 1)],
                                fT_all.ap()[g, 64:128, :])

        def lhsT(g, dup):
            w = 128 if dup else 64
            return ccin[:, 128 * g:128 * g + w]

        # Sg came from the host (tiny Gram of the own bf16 block, rounded
        # through bf16 so the k==j relu terms still cancel against Sb)
        B8 = sb.tile([128, 8], f32, tag="B8")
        SgD = sb.tile([128, 16], f32, tag="SgD")
        nc.vector.tensor_scalar_mul(B8[0:64, :], Sg[:, 0:8], -1.0)
        nc.vector.tensor_scalar_mul(B8[64:128, :], Sg[:, 8:16], -1.0)
        nc.vector.tensor_copy(SgD[0:64, :], Sg[:, :])
        nc.vector.tensor_copy(SgD[64:128, :], Sg[:, :])
        zeros = sb.tile([128, 512], BF16, tag="zeros")
        nc.vector.memset(zeros[:, :], 0.0)

        scrap_d = sb.tile([128, 512], BF16, tag="scrap_d")
        scrap_a = sb.tile([128, 512], BF16, tag="scrap_a")
        scrap_p = sb.tile([128, 16], BF16, tag="scrap_p")
        racc_d = sb.tile([128, DVE_SLOTS], f32, tag="racc_d")
        racc_a = sb.tile([128, 8 - DVE_SLOTS], f32, tag="racc_a")
        racc_p = sb.tile([128, 8], f32, tag="racc_p")

        # S slice duplicated by the PE itself: lhsT columns are the own block
        # twice, so psum partitions 0-63 and 64-127 both hold S [64, 512].
        ps_S = ps.tile([128, 512], f32, tag="ps_S")
        for g in range(4):
            nc.tensor.matmul(ps_S[:, :], lhsT(g, True),
                             rhs[:, 512 * g:512 * (g + 1)],
                             start=(g == 0), stop=(g == 3))
        # r_pos on DVE overlaps the PSUM->SBUF staging copies on ACT (Tile
        # serializes any two engines on one PSUM bank, even read-read)
        for i in range(8):
            nc.vector.scalar_tensor_tensor(
                out=scrap_p[:, :], in0=SgD[:, :], scalar=B8[:, i:i + 1],
                in1=zeros[:, 0:16], op0=ALU.add, op1=ALU.max,
                accum_out=racc_p[:, i:i + 1])
        Sb = sb.tile([128, 512], BF16, tag="Sb")
        nc.scalar.copy(Sb[0:64, :], ps_S[0:64, :])
        nc.scalar.copy(Sb[64:128, :], ps_S[64:128, :])
        for i in range(8):
            if i < DVE_SLOTS:
                nc.vector.scalar_tensor_tensor(
                    out=scrap_d[:, :], in0=Sb[:, :], scalar=B8[:, i:i + 1],
                    in1=zeros[:, :], op0=ALU.add, op1=ALU.max,
                    accum_out=racc_d[:, i:i + 1])
            else:
                nc.scalar.activation(
                    scrap_a[:, :], Sb[:, :], AF.Relu, bias=B8[:, i:i + 1],
                    accum_out=racc_a[:, i - DVE_SLOTS:i - DVE_SLOTS + 1])
        nc.sync.dma_start(out_d.ap()[:, 0:DVE_SLOTS], racc_d[:, :])
        nc.scalar.dma_start(out_d.ap()[:, DVE_SLOTS:8], racc_a[:, :])
        nc.sync.dma_start(out_d.ap()[:, 8:16], racc_p[:, :])
    nc.compile()
    return nc


_NC1 = None
_NC2 = None


def _get_ncs():
    global _NC1, _NC2
    if _NC1 is None:
        _NC1 = build_phase1()
        _NC2 = build_phase2()
    return _NC1, _NC2


def make_in_maps1(outputs, targets):
    outputs = np.ascontiguousarray(
        np.asarray(outputs, dtype=np.float32)).reshape(BATCH, FEAT, HW)
    targets = np.ascontiguousarray(
        np.asarray(targets, dtype=np.float32)).reshape(BATCH, FEAT, HW)
    return [
        {
            "x_out": np.ascontiguousarray(outputs[m * BPC:(m + 1) * BPC]),
            "x_tgt": np.ascontiguousarray(targets[m * BPC:(m + 1) * BPC]),
        }
        for m in range(NCORES)
    ]


# column permutation: branch-ordered [out b, tgt b] -> reference interleaved
# col = 16*(b//8) + 8*branch + b%8
_PERM = np.empty(64, np.int64)
for _b in range(32):
    _PERM[16 * (_b // 8) + (_b % 8)] = _b            # outputs branch
    _PERM[16 * (_b // 8) + 8 + (_b % 8)] = 32 + _b   # targets branch


def make_in_maps2(results1):
    """pooled [128(g,b), 128] bf16 per branch -> bf16 fT blocks, interleaved."""
    blocks = []
    for m in range(NCORES):
        fs = []
        for key in ("p_o", "p_t"):
            p = results1[m][key].astype(np.float32)       # [128, 128]
            v = np.concatenate([p[32 * g:32 * (g + 1), :] for g in range(4)],
                               axis=1)                    # [32 b, 512 c]
            fs.append(v / np.linalg.norm(v, axis=1, keepdims=True))
        f = np.concatenate(fs, axis=0)                    # [64 rows, 512]
        f = f[_PERM, :]                                   # reference order
        fT = f.T.reshape(4, 128, 64)                      # [g, d_local, col]
        blocks.append(fT.astype(ml_dtypes.bfloat16))
    fT_all = np.ascontiguousarray(np.concatenate(blocks, axis=2))
    maps = []
    for m in range(NCORES):
        fm = blocks[m].astype(np.float32).reshape(512, 64)  # [d, col]
        G = fm.T @ fm
        sg = np.concatenate(
            [G[16 * c:16 * (c + 1), 16 * c:16 * (c + 1)] for c in range(4)], 0)
        sg = sg.astype(ml_dtypes.bfloat16).astype(np.float32)
        maps.append({"fT_all": fT_all,
                     "fT_own": np.ascontiguousarray(
                         np.concatenate([blocks[m], blocks[m]], axis=2)),
                     "sg": np.ascontiguousarray(sg)})
    return maps


def finish(results2):
    total = 0.0
    for m in range(NCORES):
        racc = results2[m]["out"].astype(np.float64)      # [128, 16]
        total += ((1.0 + racc[:, 8:16]) / (1.0 + racc[:, 0:8])).sum()
    return np.array(1.0 - total / (GROUP * B2), dtype=np.float32)


def kernel(outputs, targets):
    nc1, nc2 = _get_ncs()
    res1 = run_bass_kernel_spmd(nc1, make_in_maps1(outputs, targets),
                                core_ids=list(range(NCORES)))
    res2 = run_bass_kernel_spmd(nc2, make_in_maps2(res1.results),
                                core_ids=list(range(NCORES)))
    return finish(res2.results)


if __name__ == "__main__":
    import reference as ref
    inputs = ref.setup_inputs()
    actual = kernel(**{k: np.asarray(v) for k, v in inputs.items()})
    print("kernel result:", actual)


# revision 24
# speedup vs baseline: 1.0063x; 1.0063x over previous
"""APKDA loss (pool+normalize -> SmoothAP) as two distributed Bass launches on
8 TRN2 NeuronCores.

Math restructuring vs the reference:
  - Only the diagonal class-blocks of sim_all_rk are ever used, so per query q
    we need rank sums only over its 16 same-class columns j:
        r_all[q,j] = 1 + sum_k relu(S[q,k] - S[q,j])   (k over all 512 columns)
        r_pos[q,j] = 1 + sum_k relu(Sg[q,k] - Sg[q,j]) (k over the 16-group)
    with Sg the own-class block of S.  The eye-masks in the reference only
    kill k==j terms whose relu is 0 anyway.
  - L2-normalizing the hw-sum equals normalizing the hw-mean (scale cancels).

Sharding: batch-parallel.  Core m owns 4 classes = outputs[32m:32m+32] and
targets[32m:32m+32] (6.4MB of the 51.4MB input).

Phase 1 (memory-bound): each core sum-pools its 6.4MB shard over the 7x7
window and returns the raw [128(g,b), 128] pooled sums per branch as bf16.
Host: normalize rows, transpose to feature-major, interleave to reference row
order, concatenate all cores -> full f^T (512KB bf16).
Phase 2: every core gets the full f^T, its own 64 columns duplicated (so
one matmul writes S onto all 128 psum partitions), and the host-computed
own-class Gram diag Sg (bf16-rounded like Sb, so the k==j relu terms cancel);
each core computes its [64, 512] slice of S and the raw rank sums; the host
applies the +1/division/total.

A single-launch variant with an in-kernel AllGather measured 133.7us:
collectives on this runtime have a ~40-80us latency floor regardless of size,
so the f exchange goes through the host instead (two NEFF launches at ~13us
fixed overhead each).  Input DMA tops out at ~205 GB/s/core here no matter
the descriptor path (sync/scalar/gpsimd) or shape, so phase 1 is pinned at
~31us of DMA + overheads.

Measured (neuron-profile exec_time_ns, core 0): phase 1 ~44.5us + phase 2
~23.5us = ~68us total (the shared machine drifts between a fast and a ~15%
slower mode), rel err vs the f32 reference 5.5e-6.  Phase 1 is at its local
floor: ~7us exec preamble + ~31us DMA at the ~205 GB/s cap + ~3us pooling
tail + ~4us Tile drain.  Remaining known headroom: replacing the two
launches with one launch + remote_dma cross-core exchange would save one
~13us launch overhead; Tile serializes any two engines touching the same
PSUM bank (even read-read), which is why S is staged to SBUF before the
rank loop.
"""

import numpy as np
import ml_dtypes

import concourse.bass as bass
import concourse.bacc as bacc
import concourse.mybir as mybir
import concourse.tile as tile
from concourse.bass_utils import run_bass_kernel_spmd

F32 = mybir.dt.float32
BF16 = mybir.dt.bfloat16
NCORES = 8
BATCH, FEAT, HW = 256, 512, 49
BPC = BATCH // NCORES          # 32 batch rows per branch per core
GROUP, B2 = 16, 512

# j-slot split of the 8 (two-j-per-instruction) rank iterations
DVE_SLOTS = 4                  # slots 0..3 on VectorE, 4..7 on ScalarE

# pooling chunk widths (c_local units); outputs loads first, targets' last
# chunk is small so the pooling tail after the final DMA is short
O_CHUNKS = [64, 64]
T_CHUNKS = [56, 56, 8, 8]


def build_phase1(dbg=None):
    """Sum-pool the shard; out: p_o / p_t bf16 [128(g,b), 128 c_local]."""
    nc = bacc.Bacc("TRN2", target_bir_lowering=False, debug=False,
                   num_devices=NCORES)
    f32 = F32
    AX = mybir.AxisListType
    x_out = nc.dram_tensor("x_out", [BPC, FEAT, HW], f32, kind="ExternalInput")
    x_tgt = nc.dram_tensor("x_tgt", [BPC, FEAT, HW], f32, kind="ExternalInput")
    po_d = nc.dram_tensor("p_o", [128, 128], BF16, kind="ExternalOutput")
    pt_d = nc.dram_tensor("p_t", [128, 128], BF16, kind="ExternalOutput")

    with tile.TileContext(nc) as tc, tc.tile_pool(name="sb", bufs=1) as sb:
        xo = sb.tile([128, 6272], f32, tag="xo")
        xt = sb.tile([128, 6272], f32, tag="xt")
        pooled_o = sb.tile([128, 128], BF16, tag="pooled_o")
        pooled_t = sb.tile([128, 128], BF16, tag="pooled_t")

        # partition p = 32g + b; row (g,b) holds x[b, 128g:128g+128, :] flat.
        # g=0,1 (partitions 0-63) ride the sync HWDGE ring, g=2,3 ride scalar,
        # which spreads the load over all 16 SDMA engines.  The reduce
        # accumulates in f32 internally and rounds once on the bf16 store, and
        # all but the last columns are shipped out before the last chunk lands.
        def load_chunks(t_, x_, p_, p_d, widths):
            c0 = 0
            for w in widths:
                for g in range(4):
                    eng = nc.sync if g < 2 else nc.scalar
                    eng.dma_start(
                        t_[32 * g:32 * (g + 1), 49 * c0:49 * (c0 + w)],
                        x_.ap()[:, g * 128 + c0:g * 128 + c0 + w, :])
                with nc.allow_low_precision("f32 accumulate, single bf16 round"):
                    nc.vector.reduce_sum(
                        p_[:, c0:c0 + w],
                        t_[:, 49 * c0:49 * (c0 + w)].rearrange(
                            "p (c h) -> p c h", h=HW),
                        axis=AX.X)
                c0 += w
            c1 = c0 - widths[-1]
            nc.sync.dma_start(p_d.ap()[0:64, 0:c1], p_[0:64, 0:c1])
            nc.scalar.dma_start(p_d.ap()[64:128, 0:c1], p_[64:128, 0:c1])
            nc.sync.dma_start(p_d.ap()[0:64, c1:c0], p_[0:64, c1:c0])
            nc.scalar.dma_start(p_d.ap()[64:128, c1:c0], p_[64:128, c1:c0])

        load_chunks(xo, x_out, pooled_o, po_d, O_CHUNKS)
        load_chunks(xt, x_tgt, pooled_t, pt_d, T_CHUNKS)
    nc.compile()
    return nc


def build_phase2(dbg=None):
    """S slice + raw rank sums from replicated bf16 f^T.
    in: fT_all [4,128,512], fT_own [4,128,128] (bf16, reference col order;
    own block's 64 columns duplicated so the PE emits S on 128 partitions);
    out: racc [128, 16] f32 (cols 0-7 r_all slots, 8-15 r_pos slots)."""
    nc = bacc.Bacc("TRN2", target_bir_lowering=False, debug=False,
                   num_devices=NCORES)
    f32 = F32
    AF = mybir.ActivationFunctionType
    ALU = mybir.AluOpType
    fT_all = nc.dram_tensor("fT_all", [4, 128, 512], BF16, kind="ExternalInput")
    fT_own = nc.dram_tensor("fT_own", [4, 128, 128], BF16, kind="ExternalInput")
    sg_in = nc.dram_tensor("sg", [64, 16], F32, kind="ExternalInput")
    out_d = nc.dram_tensor("out", [128, 16], f32, kind="ExternalOutput")

    with tile.TileContext(nc) as tc, (
            tc.tile_pool(name="sb", bufs=1)) as sb, (
            tc.tile_pool(name="ps", bufs=1, space="PSUM")) as ps:
        ccin = sb.tile([128, 512], BF16, tag="ccin")   # free = (g, col-dup)
        rhs = sb.tile([128, 2048], BF16, tag="rhs")    # free = (g, key)
        # sg first (tiny, no deps) so B8/r_pos are ready long before S;
        # own block next; even/odd partition halves ride the two HWDGE rings
        Sg = sb.tile([64, 16], F32, tag="Sg")
        nc.sync.dma_start(Sg[:, :], sg_in.ap())
        nc.sync.dma_start(
            ccin[0:64, :].rearrange("p (g n) -> p g n", g=4),
            fT_own.ap()[:, 0:64, :].rearrange("g p n -> p g n"))
        nc.scalar.dma_start(
            ccin[64:128, :].rearrange("p (g n) -> p g n", g=4),
            fT_own.ap()[:, 64:128, :].rearrange("g p n -> p g n"))
        for g in range(4):
            nc.sync.dma_start(rhs[0:64, 512 * g:512 * (g + 1)],
                              fT_all.ap()[g, 0:64, :])
            nc.scalar.dma_start(rhs[64:128, 512 * g:512 * (g + 1)],
                                fT_all.ap()[g, 64:128, :])

        def lhsT(g, dup):
            w = 128 if dup else 64
            return ccin[:, 128 * g:128 * g + w]

        # Sg came from the host (tiny Gram of the own bf16 block, rounded
        # through bf16 so the k==j relu terms still cancel against Sb)
        B8 = sb.tile([128, 8], f32, tag="B8")
        SgD = sb.tile([128, 16], f32, tag="SgD")
        nc.vector.tensor_scalar_mul(B8[0:64, :], Sg[:, 0:8], -1.0)
        nc.vector.tensor_scalar_mul(B8[64:128, :], Sg[:, 8:16], -1.0)
        nc.vector.tensor_copy(SgD[0:64, :], Sg[:, :])
        nc.vector.tensor_copy(SgD[64:128, :], Sg[:, :])
        zeros = sb.tile([128, 512], BF16, tag="zeros")
        nc.vector.memset(zeros[:, :], 0.0)

        scrap_d = sb.tile([128, 512], BF16, tag="scrap_d")
        scrap_a = sb.tile([128, 512], BF16, tag="scrap_a")
        scrap_p = sb.tile([128, 16], BF16, tag="scrap_p")
        racc_d = sb.tile([128, DVE_SLOTS], f32, tag="racc_d")
        racc_a = sb.tile([128, 8 - DVE_SLOTS], f32, tag="racc_a")
        racc_p = sb.tile([128, 8], f32, tag="racc_p")

        # S slice duplicated by the PE itself: lhsT columns are the own block
        # twice, so psum partitions 0-63 and 64-127 both hold S [64, 512].
        # Two key-half psum tiles = two banks, so the two staging copies
        # below can run on ACT and DVE concurrently (Tile serializes any two
        # engines on one PSUM bank, even read-read).
        ps_S = [ps.tile([128, 256], f32, tag=f"ps_S{h}", name=f"psS{h}")
                for h in range(2)]
        for h in range(2):
            for g in range(4):
                nc.tensor.matmul(
                    ps_S[h][:, :], lhsT(g, True),
                    rhs[:, 512 * g + 256 * h:512 * g + 256 * (h + 1)],
                    start=(g == 0), stop=(g == 3))
        # r_pos on DVE runs in the DMA/matmul shadow
        for i in range(8):
            nc.vector.scalar_tensor_tensor(
                out=scrap_p[:, :], in0=SgD[:, :], scalar=B8[:, i:i + 1],
                in1=zeros[:, 0:16], op0=ALU.add, op1=ALU.max,
                accum_out=racc_p[:, i:i + 1])
        Sb = sb.tile([128, 512], BF16, tag="Sb")
        nc.scalar.copy(Sb[:, 0:256], ps_S[0][:, :])
        nc.vector.tensor_copy(Sb[:, 256:512], ps_S[1][:, :])
        for i in range(8):
            if i < DVE_SLOTS:
                nc.vector.scalar_tensor_tensor(
                    out=scrap_d[:, :], in0=Sb[:, :], scalar=B8[:, i:i + 1],
                    in1=zeros[:, :], op0=ALU.add, op1=ALU.max,
                    accum_out=racc_d[:, i:i + 1])
            else:
                nc.scalar.activation(
                    scrap_a[:, :], Sb[:, :], AF.Relu, bias=B8[:, i:i + 1],
                    accum_out=racc_a[:, i - DVE_SLOTS:i - DVE_SLOTS + 1])
        nc.sync.dma_start(out_d.ap()[:, 0:DVE_SLOTS], racc_d[:, :])
        nc.scalar.dma_start(out_d.ap()[:, DVE_SLOTS:8], racc_a[:, :])
        nc.sync.dma_start(out_d.ap()[:, 8:16], racc_p[:, :])
    nc.compile()
    return nc


_NC1 = None
_NC2 = None


def _get_ncs():
    global _NC1, _NC2
    if _NC1 is None:
        _NC1 = build_phase1()
        _NC2 = build_phase2()
    return _NC1, _NC2


def make_in_maps1(outputs, targets):
    outputs = np.ascontiguousarray(
        np.asarray(outputs, dtype=np.float32)).reshape(BATCH, FEAT, HW)
    targets = np.ascontiguousarray(
        np.asarray(targets, dtype=np.float32)).reshape(BATCH, FEAT, HW)
    return [
        {
            "x_out": np.ascontiguousarray(outputs[m * BPC:(m + 1) * BPC]),
            "x_tgt": np.ascontiguousarray(targets[m * BPC:(m + 1) * BPC]),
        }
        for m in range(NCORES)
    ]


# column permutation: branch-ordered [out b, tgt b] -> reference interleaved
# col = 16*(b//8) + 8*branch + b%8
_PERM = np.empty(64, np.int64)
for _b in range(32):
    _PERM[16 * (_b // 8) + (_b % 8)] = _b            # outputs branch
    _PERM[16 * (_b // 8) + 8 + (_b % 8)] = 32 + _b   # targets branch


def make_in_maps2(results1):
    """pooled [128(g,b), 128] bf16 per branch -> bf16 fT blocks, interleaved."""
    blocks = []
    for m in range(NCORES):
        fs = []
        for key in ("p_o", "p_t"):
            p = results1[m][key].astype(np.float32)       # [128, 128]
            v = np.concatenate([p[32 * g:32 * (g + 1), :] for g in range(4)],
                               axis=1)                    # [32 b, 512 c]
            fs.append(v / np.linalg.norm(v, axis=1, keepdims=True))
        f = np.concatenate(fs, axis=0)                    # [64 rows, 512]
        f = f[_PERM, :]                                   # reference order
        fT = f.T.reshape(4, 128, 64)                      # [g, d_local, col]
        blocks.append(fT.astype(ml_dtypes.bfloat16))
    fT_all = np.ascontiguousarray(np.concatenate(blocks, axis=2))
    maps = []
    for m in range(NCORES):
        fm = blocks[m].astype(np.float32).reshape(512, 64)  # [d, col]
        G = fm.T @ fm
        sg = np.concatenate(
            [G[16 * c:16 * (c + 1), 16 * c:16 * (c + 1)] for c in range(4)], 0)
        sg = sg.astype(ml_dtypes.bfloat16).astype(np.float32)
        maps.append({"fT_all": fT_all,
                     "fT_own": np.ascontiguousarray(
                         np.concatenate([blocks[m], blocks[m]], axis=2)),
                     "sg": np.ascontiguousarray(sg)})
    return maps


def finish(results2):
    total = 0.0
    for m in range(NCORES):
        racc = results2[m]["out"].astype(np.float64)      # [128, 16]
        total += ((1.0 + racc[:, 8:16]) / (1.0 + racc[:, 0:8])).sum()
    return np.array(1.0 - total / (GROUP * B2), dtype=np.float32)


def kernel(outputs, targets):
    nc1, nc2 = _get_ncs()
    res1 = run_bass_kernel_spmd(nc1, make_in_maps1(outputs, targets),
                                core_ids=list(range(NCORES)))
    res2 = run_bass_kernel_spmd(nc2, make_in_maps2(res1.results),
                                core_ids=list(range(NCORES)))
    return finish(res2.results)


if __name__ == "__main__":
    import reference as ref
    inputs = ref.setup_inputs()
    actual = kernel(**{k: np.asarray(v) for k, v in inputs.items()})
    print("kernel result:", actual)
